# revision 53
# baseline (speedup 1.0000x reference)
"""CondConv kernel for Trainium2 (8 NeuronCores, data-parallel over batch).

Computation (per sample b):
  att   = sigmoid(mean_hw(x_b) @ att_w.T)                       [K]
  agg_w = sum_k att[k] * weight[k]    (3x3 conv weights, O,I)   [O,I,3,3]
  out   = BN(conv2d(x_b, agg_w, pad=1) + att @ bias) -> relu    [O,H,W]

Strategy: pure data parallel, 4 samples per core.  Two builders:

* build_program (fallback, WINO=False): direct conv as 9 shifted bf16
  matmuls (contraction over C_in on partitions, 2 c-tiles) accumulating
  in PSUM, spatial dim in row-blocks of 8 (N=448).
* build_program_wino (default): 1-D row Winograd F(2,3) in fp16 --
  G-transform of the expert bank on the host, B^T row transform of x on
  DVE/Pool, 12 transform-domain taps (4 coords x 3 kw) on the PE (2/3
  the direct-conv PE work), A^T output stage on Pool, bias/relu on DVE,
  ACT draining PSUM, y written fp16 and upcast on the host.  See the
  builder docstring for the engine balance.

Common to both: BN scale folded into weights on the host; BN shift +
conv bias fold into the drain; all bulk DMA on HWDGE via the SP queue
(x0 first, then the weight bank a-major so sample-0 combine chunks
unblock in arrival order); next-sample prep (x load / transform / att /
weight-combine) interleaved mid-conv so the PE never idles at sample
boundaries.  Deeper-transform variants (F(4,3), 2-D F(2x2)) were
analyzed and rejected: under this cost model every further PE-row
reduction adds more elementwise transform work than DVE+Pool capacity
(~48us/sample-pair) can absorb.
"""

from contextlib import ExitStack
from itertools import product

import ml_dtypes
import numpy as np

import concourse.bass as bass
import concourse.mybir as mybir
import concourse.tile as tile
from concourse import bacc, bass_isa
from concourse.bass_utils import run_bass_kernel_spmd

BS, C_IN, H, W = 32, 256, 56, 56
C_OUT, K_EXP = 256, 4
NCORES = 8
BPC = BS // NCORES          # samples per core
CT = C_IN // 128            # input-channel tiles
OT = C_OUT // 128           # output-channel tiles
R = 8                       # output rows per psum block

F32 = mybir.dt.float32
BF16 = mybir.dt.bfloat16
AF = mybir.ActivationFunctionType
ALU = mybir.AluOpType


def build_program(bpc=BPC, h=H, w=W, reps=1):
    """Build the per-core SPMD program (identical on all cores).

    reps>1 unrolls the whole body N times (timing-only: lets wall-clock
    differences measure per-iteration device time past dispatch overhead).
    """
    assert h % R == 0
    nblk = h // R
    n = R * w              # matmul free dim per block
    hp = h + 2             # padded rows (row 0 and hp-1 stay zero)
    wpr = w + 4            # padded row pitch (data at cols 2..w+1)
    hw = h * w

    nc = bacc.Bacc(
        "TRN2", target_bir_lowering=False, debug=False, enable_asserts=False
    )

    x_d = nc.declare_dram_parameter("x", [bpc, C_IN, h, w], F32, isOutput=False)
    wt_d = nc.declare_dram_parameter(
        "wt", [128, CT, 3, K_EXP, 3, C_OUT], BF16, isOutput=False
    )
    awt_d = nc.declare_dram_parameter("att_wt", [128, CT, K_EXP], F32, isOutput=False)
    bias_d = nc.declare_dram_parameter(
        "biasT", [128, OT, K_EXP], F32, isOutput=False
    )
    inv_d = nc.declare_dram_parameter("bninv", [C_OUT], F32, isOutput=False)
    cnst_d = nc.declare_dram_parameter("bncnst", [C_OUT], F32, isOutput=False)
    y_d = nc.declare_dram_parameter("y", [bpc, C_OUT, hw], F32, isOutput=True)

    with ExitStack() as ctx:
        tc = ctx.enter_context(tile.TileContext(nc))
        cpool = ctx.enter_context(tc.tile_pool(name="consts", bufs=1))
        wpool = ctx.enter_context(tc.tile_pool(name="work", bufs=2))
        ppool = ctx.enter_context(
            tc.tile_pool(name="psumc", bufs=nblk, space=bass.MemorySpace.PSUM)
        )
        # ---- per-sample state ----------------------------------------------
        # pooled partial sums: one column per (sample, ct, row-half); the
        # half-granularity lets casts pipeline behind the x DMA halves.
        pooled2 = cpool.tile([128, bpc, CT, 2], F32, tag="pooled2", name="pooled2")
        pooled = cpool.tile([128, bpc * CT], F32, tag="pooled", name="pooled")
        att_bc = cpool.tile([128, bpc, K_EXP], F32, tag="attbc", name="att_bc")
        bias_tot = cpool.tile([128, bpc, OT], F32, tag="btot", name="bias_tot")
        xstage = {}
        xpad = {}
        aggs = {}
        atts = {}

        hh = h // 2

        def prep_load(b, skip_dma=False):
            """x DMA (HWDGE via SP, row-halves) + cast-to-padded-bf16 +
            pooled partial sums.  Half granularity pipelines each cast
            behind its DMA half instead of the whole chunk."""
            if not skip_dma:
                for ct in range(CT):
                    xs = wpool.tile(
                        [128, h, w], F32, tag="xstage", bufs=4, name=f"xs_{b}_{ct}"
                    )
                    for hf in range(2):
                        nc.sync.dma_start(
                            out=xs[:, hf * hh : (hf + 1) * hh],
                            in_=x_d[
                                b, ct * 128 : (ct + 1) * 128, hf * hh : (hf + 1) * hh
                            ],
                        )
                    xstage[b, ct] = xs
            for ct in range(CT):
                for hf in range(2):
                    nc.scalar.activation(
                        xpad[b, ct][:, 1 + hf * hh : 1 + (hf + 1) * hh, 2 : w + 2],
                        xstage[b, ct][:, hf * hh : (hf + 1) * hh],
                        AF.Copy,
                        accum_out=pooled2[:, b, ct, hf : hf + 1],
                    )
            nc.vector.tensor_add(
                pooled[:, b * CT : (b + 1) * CT],
                pooled2[:, b, :, 0],
                pooled2[:, b, :, 1],
            )

        # sample-0 x loads first (front of the DMA queues)
        for ct in range(CT):
            xs = wpool.tile([128, h, w], F32, tag="xstage", bufs=4, name=f"xs_0_{ct}")
            for hf in range(2):
                nc.sync.dma_start(
                    out=xs[:, hf * hh : (hf + 1) * hh],
                    in_=x_d[0, ct * 128 : (ct + 1) * 128, hf * hh : (hf + 1) * hh],
                )
            xstage[0, ct] = xs

        def prep_load0():
            """casts for sample 0 (x DMAs were issued up front)."""
            prep_load(0, skip_dma=True)

        # ---- resident constants --------------------------------------------
        # att weights first (tiny, gate the attention head), then wt chunks
        # behind sample-0's x on the HWDGE path; (ct, kh)-chunking lets the
        # weight-combine (and the conv) start before the whole bank arrives.
        awt_sb = cpool.tile([128, CT, K_EXP], F32, tag="awt", name="awt_sb")
        nc.sync.dma_start(out=awt_sb[:], in_=awt_d[:])
        bias_sb = cpool.tile([128, OT, K_EXP], F32, tag="bias", name="bias_sb")
        nc.sync.dma_start(out=bias_sb[:], in_=bias_d[:])
        wt_sb = cpool.tile(
            [128, CT, 3, K_EXP, 3, C_OUT], BF16, tag="wt", name="wt_sb"
        )
        for ct, kh in product(range(CT), range(3)):
            nc.sync.dma_start(out=wt_sb[:, ct, kh], in_=wt_d[:, ct, kh])
        inv_sb = cpool.tile([128, OT], F32, tag="inv", name="inv_sb")
        nc.sync.dma_start(out=inv_sb[:], in_=inv_d[:].rearrange("(t p) -> p t", p=128))
        cnst_sb = cpool.tile([128, OT], F32, tag="cnst", name="cnst_sb")
        nc.sync.dma_start(
            out=cnst_sb[:], in_=cnst_d[:].rearrange("(t p) -> p t", p=128)
        )
        ones_bf = cpool.tile([K_EXP, 128], BF16, tag="onesbf", name="ones_bf")
        nc.vector.memset(ones_bf[:], 1.0)
        # Dummy sigmoid as the first ACT op: pins the "sigmoid_and_others"
        # table set (which also holds Copy and Relu) so the per-sample
        # sigmoid never pays a mid-kernel 1.3us LoadActFuncSet switch.
        wact = cpool.tile([1, 2], F32, tag="wact", name="warm_act")
        nc.vector.memset(wact[:], 0.0)
        nc.scalar.activation(wact[:, 0:1], wact[:, 1:2], AF.Sigmoid)
        for b, ct in product(range(bpc), range(CT)):
            t = cpool.tile(
                [128, hp, wpr], BF16, tag=f"xpad_{b}_{ct}", name=f"xpad_{b}_{ct}"
            )
            # zero only the borders (interior is fully overwritten);
            # all on GpSimd so the DVE stream stays clear for the combine
            nc.gpsimd.memset(t[:, 0:1, :], 0.0)
            nc.gpsimd.memset(t[:, hp - 1 : hp, :], 0.0)
            nc.gpsimd.memset(t[:, :, 0:2], 0.0)
            nc.gpsimd.memset(t[:, :, wpr - 2 : wpr], 0.0)
            xpad[b, ct] = t

        def prep_att_head(b):
            """attention for sample b, PE-free: per-partition products on
            DVE, cross-partition sum on GpSimd (result lands replicated on
            all partitions), sigmoid on ACT."""
            lg = wpool.tile([128, K_EXP], F32, tag="lgt", name=f"lgt_{b}")
            nc.vector.tensor_scalar_mul(
                lg[:], awt_sb[:, 0, :], pooled[:, b * CT : b * CT + 1]
            )
            nc.vector.scalar_tensor_tensor(
                lg[:],
                awt_sb[:, 1, :],
                pooled[:, b * CT + 1 : b * CT + 2],
                lg[:],
                op0=ALU.mult,
                op1=ALU.add,
            )
            red = wpool.tile([128, K_EXP], F32, tag="lgr", name=f"lgr_{b}")
            nc.gpsimd.partition_all_reduce(
                red[:], lg[:], 128, bass_isa.ReduceOp.add
            )
            nc.scalar.activation(att_bc[:, b, :], red[:], AF.Sigmoid, scale=1.0 / hw)

        def prep_att_combine(b):
            # agg = sum_k att[k] * wt[k], sub-chunked by (ct, kh) in conv
            # tap order, all on DVE (ts_mul runs 4x, tensor_add 2x bf16).
            # One tile per chunk keeps the dep tracking fine-grained: the
            # conv can start as soon as its first chunk is combined.
            agg = {}
            for ct, kh in product(range(CT), range(3)):
                a = wpool.tile(
                    [128, 3, C_OUT], BF16, tag="agg", bufs=12,
                    name=f"agg_{b}_{ct}{kh}",
                )
                nc.vector.tensor_scalar_mul(
                    a[:], wt_sb[:, ct, kh, 0], att_bc[:, b, 0:1]
                )
                for k in range(1, K_EXP):
                    tm = wpool.tile(
                        [128, 3, C_OUT], BF16, tag="tm", bufs=2,
                        name=f"tm_{b}{ct}{kh}{k}",
                    )
                    nc.vector.tensor_scalar_mul(
                        tm[:], wt_sb[:, ct, kh, k], att_bc[:, b, k : k + 1]
                    )
                    nc.vector.tensor_add(a[:], a[:], tm[:])
                agg[ct, kh] = a
            aggs[b] = agg

        def conv_ot(b, ot):
            """one output-channel tile of the conv for sample b."""
            agg = aggs[b]
            # bias_tot[o, b] = (sum_k att[k] bias[k, o]) * inv + cnst —
            # PE-free; only the drains (~25us later) need it
            pb = wpool.tile([128, K_EXP], F32, tag="pb", bufs=2, name=f"pb_{b}_{ot}")
            nc.vector.tensor_mul(pb[:], bias_sb[:, ot, :], att_bc[:, b, :])
            pbr = wpool.tile([128, 1], F32, tag="pbr", bufs=2, name=f"pbr_{b}_{ot}")
            nc.vector.tensor_reduce(
                pbr[:], pb[:], axis=mybir.AxisListType.X, op=ALU.add
            )
            nc.vector.tensor_scalar(
                bias_tot[:, b, ot : ot + 1],
                pbr[:],
                inv_sb[:, ot : ot + 1],
                cnst_sb[:, ot : ot + 1],
                op0=ALU.mult,
                op1=ALU.add,
            )
            taps = list(product(range(CT), range(3), range(3)))
            # two waves of psum blocks: wave A's drains overlap wave B's
            # accumulation, so the next group never waits on bank turnaround.
            # 5+2 keeps the final wave's post-matmul drain tail short.
            half = min(5, nblk)
            for wave in (range(0, half), range(half, nblk)):
                wave = list(wave)
                if not wave:
                    continue
                ps = {
                    blk: ppool.tile([128, n], F32, tag="cps", name=f"ps_{b}_{ot}_{blk}")
                    for blk in wave
                }
                for ci, (ct, kh, kw) in enumerate(taps):
                    lhsT = agg[ct, kh][:, kw, ot * 128 : (ot + 1) * 128]
                    for blk in wave:
                        nc.tensor.matmul(
                            ps[blk][:],
                            lhsT,
                            xpad[b, ct][
                                :, blk * R + kh : blk * R + kh + R, 1 + kw : 1 + kw + w
                            ],
                            start=(ci == 0),
                            stop=(ci == len(taps) - 1),
                        )
                # drain: relu(psum + bias_tot), alternating ACT/DVE
                for blk in wave:
                    osb = wpool.tile(
                        [128, n], F32, tag="osb", bufs=6, name=f"osb_{b}_{ot}_{blk}"
                    )
                    if blk % 2 == 0:
                        nc.scalar.activation(
                            osb[:],
                            ps[blk][:],
                            AF.Relu,
                            bias=bias_tot[:, b, ot : ot + 1],
                        )
                    else:
                        nc.vector.tensor_scalar(
                            osb[:],
                            ps[blk][:],
                            bias_tot[:, b, ot : ot + 1],
                            0.0,
                            op0=ALU.add,
                            op1=ALU.max,
                        )
                    nc.sync.dma_start(
                        out=y_d[b, ot * 128 : (ot + 1) * 128, blk * n : (blk + 1) * n],
                        in_=osb[:],
                    )

        # ---- main schedule --------------------------------------------------
        for rep in range(reps):
            if rep == 0:
                prep_load0()
            else:
                prep_load(0)

            # PE warm-up: junk matmuls (gated on the first cast) keep HAM
            # from clocking the PE at 1.2 GHz at the first conv matmuls.
            if rep == 0:
                warm = spool.tile([128, n], F32, tag="sps", name="warm_ps")
                for i in range(8):
                    nc.tensor.matmul(
                        warm[:],
                        ones_bf[:, 0:128],
                        xpad[0, 0][0:K_EXP, 1 : 1 + R, 2 : 2 + w],
                        start=True,
                        stop=True,
                    )

            prep_att_head(0)
            prep_att_combine(0)
            for b in range(bpc):
                if b + 1 < bpc:
                    prep_load(b + 1)
                    prep_att_head(b + 1)
                    prep_att_combine(b + 1)
                conv_ot(b, 0)
                conv_ot(b, 1)
    nc.compile()
    return nc


def host_inputs(inputs, bpc=BPC, h=H, w=W):
    """Shard x over batch; lay out replicated params for the device."""
    x = np.ascontiguousarray(np.asarray(inputs["x"], dtype=np.float32))
    att_w = np.asarray(inputs["att_w"], dtype=np.float32)
    weight = np.asarray(inputs["weight"], dtype=np.float32)
    bias = np.asarray(inputs["bias"], dtype=np.float32)

    # Fold the BN scale inv = gamma/sqrt(var+eps) into the conv weights so
    # the PSUM drain is a single relu(psum + bias_tot) op.  (bias_tot gets
    # its inv factor on-device.)
    inv = np.asarray(inputs["gamma"], dtype=np.float32) / np.sqrt(
        np.asarray(inputs["run_var"], dtype=np.float32) + 1e-5
    )
    cnst = np.asarray(inputs["beta"], dtype=np.float32) - (
        np.asarray(inputs["run_mean"], dtype=np.float32) * inv
    )
    weight = weight * inv[None, :, None, None, None]

    # wt[i_lo, ct, kh, k, kw, o] = weight[k, o, ct*128+i_lo, kh, kw]
    wt = weight.reshape(K_EXP, C_OUT, CT, 128, 3, 3)
    wt = wt.transpose(3, 2, 4, 0, 5, 1).reshape(128, CT, 3, K_EXP, 3, C_OUT)
    wt = np.ascontiguousarray(wt).astype(ml_dtypes.bfloat16)
    # att_wt[c_lo, ct, k] = att_w[k, ct*128+c_lo]
    awt = np.ascontiguousarray(
        att_w.T.reshape(CT, 128, K_EXP).transpose(1, 0, 2)
    ).astype(np.float32)

    biasT = np.ascontiguousarray(
        bias.T.reshape(OT, 128, K_EXP).transpose(1, 0, 2)
    ).astype(np.float32)
    common = {
        "wt": wt,
        "att_wt": awt,
        "biasT": biasT,
        "bninv": inv.astype(np.float32),
        "bncnst": cnst.astype(np.float32),
    }
    return [
        {"x": x[c * bpc : (c + 1) * bpc], **common} for c in range(x.shape[0] // bpc)
    ]


F16 = mybir.dt.float16
NT = H // 2                       # winograd F(2,3) row-tile count
WBLK = [(0, 7), (7, 7), (14, 7), (21, 7)]  # (tile-row start, count) psum blocks


def build_program_wino(bpc=BPC, h=H, w=W, use_gpsimd_tt=True):
    """1-D row-Winograd F(2,3) variant: conv = A^T [ (G W) . (B^T x) ] along
    rows, direct 3-tap conv along columns.  PE work drops 3->2 MACs/output
    row-tap (12 taps of 4 transform coords x 3 kw vs 18 direct).

    Engine split (driven by the CoreSim cost model; measured busy at
    142.1us span: PE 129.2 (91%) / Pool 94.6 / DVE 82.8 / ACT 68.3 /
    SP 60.1; remaining PE idle is the DMA-stream-bound fill plus the
    final DMA-semaphore tail):
      * x uploaded fp16 into contiguous staging (strided DMA destinations
        shrink the contiguous runs below 512B and double the billed DMA
        latency); the row transform reads x rows directly with two tiny
        edge ops standing in for the top/bottom halo.
      * pooling for the attention head rides u1: u1 = x[2t]+x[2t+1]
        touches every x row exactly once, so a 4x-mode tensor_scalar
        accum_out pass over u1 yields sum_hw(x) almost free.  (A
        tensor_tensor_reduce would do it in one op but faults the HW
        exec unit.)
      * DVE: u0/u1, combine muls + first pair-add (TSP runs 4x, TT 2x on
        packed fp16), fused bias+relu TSPs.
      * Pool (gpsimd): u2/u3, combine second pair-add chain, the whole
        A^T stage (s/y0/d/y1) -- its TT costs ~0.83ns/elem and the
        engine is otherwise idle.
      * ACT: PSUM drains + sigmoid only.
      * PE: uniform 7-tile psum blocks (the cost model bills output rows;
        ragged blocks waste billed rows); sample 0 runs both output tiles
        coord-interleaved [ot0-a0, ot1-a0, ot0-a1, ...] so each coord
        supplies ~10us of matmul work while the next coord's weight
        chunks stream in (zero weight-wait stalls); later samples use
        coord-major order on their first output tile so the conv can
        start on the a01 agg slices; a ~4.7us warm-up chain gated on
        u1-ct1 finishes the 2x pstate ramp inside the fill gap (distinct
        rhs slices dodge dedup, writing the first live psum tile dodges
        dead-code elimination); and a split final block shortens the
        drain->A^T->relu->DMA tail.
    """
    nt = h // 2
    hp = h + 2
    wpr = w + 4
    hw = h * w

    nc = bacc.Bacc(
        "TRN2", target_bir_lowering=False, debug=False, enable_asserts=False
    )

    x_d = nc.declare_dram_parameter("x", [bpc, C_IN, h, w], F16, isOutput=False)
    wt_d = nc.declare_dram_parameter(
        "wtw", [128, CT, 4, K_EXP, 3, C_OUT], F16, isOutput=False
    )
    awt_d = nc.declare_dram_parameter("att_wt", [128, CT, K_EXP], F32, isOutput=False)
    bias_d = nc.declare_dram_parameter("biasT", [128, OT, K_EXP], F32, isOutput=False)
    inv_d = nc.declare_dram_parameter("bninv", [C_OUT], F32, isOutput=False)
    cnst_d = nc.declare_dram_parameter("bncnst", [C_OUT], F32, isOutput=False)
    y_d = nc.declare_dram_parameter("y", [bpc, C_OUT, h, w], F16, isOutput=True)

    with ExitStack() as ctx:
        tc = ctx.enter_context(tile.TileContext(nc))
        cpool = ctx.enter_context(tc.tile_pool(name="consts", bufs=1))
        wpool = ctx.enter_context(tc.tile_pool(name="work", bufs=2))
        ppool = ctx.enter_context(
            tc.tile_pool(name="psumc", bufs=8, space=bass.MemorySpace.PSUM)
        )
        spool = ctx.enter_context(
            tc.tile_pool(name="psums", bufs=1, space=bass.MemorySpace.PSUM)
        )

        pooled = cpool.tile([128, bpc * CT], F32, tag="pooled", name="pooled")
        att_bc = cpool.tile([128, bpc, K_EXP], F32, tag="attbc", name="att_bc")
        bias_tot = cpool.tile([128, bpc, OT], F32, tag="btot", name="bias_tot")

        # persistent double-buffered (b%2) staging; u borders zeroed once
        xstage = {}
        ut = {}
        aggs = {}
        for s in range(2):
            xstage[s] = cpool.tile(
                [128, CT, h, w], F16, tag=f"xs_{s}", name=f"xs_{s}"
            )
        for s, ct in product(range(2), range(CT)):
            u = cpool.tile(
                [128, 4, nt, wpr], F16, tag=f"u_{s}_{ct}", name=f"u_{s}_{ct}"
            )
            nc.gpsimd.memset(u[:, :, :, 0:2], 0.0)
            nc.gpsimd.memset(u[:, :, :, wpr - 2 : wpr], 0.0)
            ut[s, ct] = u
            aggs[s, ct] = cpool.tile(
                [128, 4, 3, C_OUT], F16, tag=f"agg_{s}_{ct}", name=f"agg_{s}_{ct}"
            )

        hh = h // 2

        def prep_load(b, skip_dma=False):
            # x DMA (fp16, HWDGE via SP): one contiguous chunk per (b, ct).
            # A strided destination would shrink the DMA's contiguous runs to
            # 112B and double its billed latency, so x stays packed and the
            # row transform below handles the top/bottom halo with two tiny
            # edge ops instead of padded rows.
            if not skip_dma:
                for ct in range(CT):
                    nc.sync.dma_start(
                        out=xstage[b % 2][:, ct],
                        in_=x_d[b, ct * 128 : (ct + 1) * 128],
                    )
            # row transform u0..u3 = B^T x over row pairs (fp16 2x TT), into
            # the u tiles' interior columns (borders are persistent zeros).
            # In x-row terms (tile t): u0=x[2t-1]-x[2t+1], u1=x[2t]+x[2t+1],
            # u2=x[2t+1]-x[2t], u3=x[2t]-x[2t+2]; u1 covers every x row once,
            # so a 4x-mode TSP pass over it accumulates sum_hw(x) for the
            # attention head.
            pool_eng = nc.gpsimd if use_gpsimd_tt else nc.vector

            def u_front(ct, u0_eng):
                """u1 (+pooled accum) and u0 for one ct; u0 on `u0_eng`."""
                xs = xstage[b % 2][:, ct]
                u = ut[b % 2, ct]
                xr = lambda r0, n: xs[:, r0 : r0 + 2 * n - 1 : 2]
                ui = lambda a, t0, n: u[:, a, t0 : t0 + n, 2 : w + 2]
                nc.vector.tensor_add(ui(1, 0, nt), xr(0, nt), xr(1, nt))
                nc.vector.tensor_scalar(
                    ui(1, 0, nt),
                    ui(1, 0, nt),
                    0.0,
                    0.0,
                    op0=ALU.add,
                    op1=ALU.add,
                    accum_out=pooled[:, b * CT + ct : b * CT + ct + 1],
                )
                nc.vector.tensor_scalar(
                    ui(0, 0, 1), xs[:, 1:2], -1.0, None, op0=ALU.mult
                )
                u0_eng.tensor_sub(ui(0, 1, nt - 1), xr(1, nt - 1), xr(3, nt - 1))

            def u_back(ct):
                """u2/u3 for one ct (Pool)."""
                xs = xstage[b % 2][:, ct]
                u = ut[b % 2, ct]
                xr = lambda r0, n: xs[:, r0 : r0 + 2 * n - 1 : 2]
                ui = lambda a, t0, n: u[:, a, t0 : t0 + n, 2 : w + 2]
                pool_eng.tensor_sub(ui(2, 0, nt), xr(1, nt), xr(0, nt))
                pool_eng.tensor_sub(ui(3, 0, nt - 1), xr(0, nt - 1), xr(2, nt - 1))
                nc.vector.tensor_copy(ui(3, nt - 1, 1), xs[:, 2 * nt - 2 : 2 * nt - 1])

            if b == 0:
                # latency-critical path: u1/u0 on DVE for both cts, the
                # attention head next (its gpsimd all_reduce must not queue
                # behind Pool u ops), u2/u3 afterwards
                for ct in range(CT):
                    u_front(ct, nc.vector)
                prep_att_head(0)
                for ct in range(CT):
                    u_back(ct)
            else:
                for ct in range(CT):
                    u_front(ct, nc.vector)
                    u_back(ct)

        # front of the DMA queue: x0-ct0 first (longest dependency chain),
        # tiny att/bias params slotted behind it, x0-ct1, then the weight
        # bank a-major so coord-0 chains get weights first
        awt_sb = cpool.tile([128, CT, K_EXP], F32, tag="awt", name="awt_sb")
        bias_sb = cpool.tile([128, OT, K_EXP], F32, tag="bias", name="bias_sb")
        nc.sync.dma_start(out=xstage[0][:, 0], in_=x_d[0, 0:128])
        nc.sync.dma_start(out=awt_sb[:], in_=awt_d[:])
        nc.sync.dma_start(out=bias_sb[:], in_=bias_d[:])
        nc.sync.dma_start(out=xstage[0][:, 1], in_=x_d[0, 128:256])
        wt_sb = cpool.tile([128, CT, 4, K_EXP, 3, C_OUT], F16, tag="wt", name="wt_sb")
        for a, ct in product(range(2), range(CT)):
            nc.sync.dma_start(out=wt_sb[:, ct, a], in_=wt_d[:, ct, a])
        for a, ct in product((2, 3), range(CT)):
            nc.sync.dma_start(out=wt_sb[:, ct, a], in_=wt_d[:, ct, a])
        inv_sb = cpool.tile([128, OT], F32, tag="inv", name="inv_sb")
        nc.sync.dma_start(out=inv_sb[:], in_=inv_d[:].rearrange("(t p) -> p t", p=128))
        cnst_sb = cpool.tile([128, OT], F32, tag="cnst", name="cnst_sb")
        nc.sync.dma_start(
            out=cnst_sb[:], in_=cnst_d[:].rearrange("(t p) -> p t", p=128)
        )
        ones_f16 = cpool.tile([128, 128], F16, tag="wones", name="ones_f16")
        nc.vector.memset(ones_f16[:], 1.0)
        wact = cpool.tile([1, 2], F32, tag="wact", name="warm_act")
        nc.vector.memset(wact[:], 0.0)
        nc.scalar.activation(wact[:, 0:1], wact[:, 1:2], AF.Sigmoid)

        def prep_att_head(b):
            lg = wpool.tile([128, K_EXP], F32, tag="lgt", name=f"lgt_{b}")
            nc.vector.tensor_scalar_mul(
                lg[:], awt_sb[:, 0, :], pooled[:, b * CT : b * CT + 1]
            )
            nc.vector.scalar_tensor_tensor(
                lg[:],
                awt_sb[:, 1, :],
                pooled[:, b * CT + 1 : b * CT + 2],
                lg[:],
                op0=ALU.mult,
                op1=ALU.add,
            )
            red = wpool.tile([128, K_EXP], F32, tag="lgr", name=f"lgr_{b}")
            nc.gpsimd.partition_all_reduce(red[:], lg[:], 128, bass_isa.ReduceOp.add)
            nc.scalar.activation(att_bc[:, b, :], red[:], AF.Sigmoid, scale=1.0 / hw)

        def prep_combine(b, fine):
            """agg[ct][:, a] = sum_k att[k] * wtw[ct, a, k]; fine=per-(ct,a)
            chunks (low latency, sample 0), else one chunk per ct."""
            pool_eng = nc.gpsimd if use_gpsimd_tt else nc.vector
            if fine:
                # serial mul/add chain in per-(a, ct) chunks: lowest latency
                # to the first usable agg slice while wt still streams in
                for (a0, na), ct in [((a, 1), ct) for a in range(4) for ct in range(CT)]:
                    dst = aggs[b % 2, ct][:, a0 : a0 + na]
                    nc.vector.tensor_scalar_mul(
                        dst, wt_sb[:, ct, a0 : a0 + na, 0], att_bc[:, b, 0:1]
                    )
                    for k in range(1, K_EXP):
                        tm = wpool.tile(
                            [128, 4, 3, C_OUT], F16, tag="tm", bufs=2,
                            name=f"tm_{b}{ct}{a0}{k}",
                        )
                        nc.vector.tensor_scalar_mul(
                            tm[:, :na], wt_sb[:, ct, a0 : a0 + na, k], att_bc[:, b, k : k + 1]
                        )
                        nc.vector.tensor_add(dst, dst, tm[:, :na])
            else:
                # tree combine per ct: all four muls on DVE (4x TSP), pair
                # sums split DVE/Pool so the chain latency halves and DVE
                # sheds the heavy full-bank adds
                # a-pair chunks, a01 for both cts first: the next sample's
                # coord-major conv needs only coord 0/1 aggs to start, so
                # the a01 slices landing early lets it dispatch right as the
                # previous sample's last chain retires
                for (p0, np_), ct in product(((0, 2), (2, 2)), range(CT)):
                    dst = aggs[b % 2, ct][:, p0 : p0 + np_]
                    tm1 = wpool.tile(
                        [128, 2, 3, C_OUT], F16, tag="tm", bufs=2,
                        name=f"tm_{b}{ct}{p0}1",
                    )
                    tm2 = wpool.tile(
                        [128, 2, 3, C_OUT], F16, tag="tmc", bufs=2,
                        name=f"tm_{b}{ct}{p0}2",
                    )
                    tm3 = wpool.tile(
                        [128, 2, 3, C_OUT], F16, tag="tm", bufs=2,
                        name=f"tm_{b}{ct}{p0}3",
                    )
                    wts = wt_sb[:, ct, p0 : p0 + np_]
                    nc.vector.tensor_scalar_mul(dst, wts[:, :, 0], att_bc[:, b, 0:1])
                    nc.vector.tensor_scalar_mul(tm1[:], wts[:, :, 1], att_bc[:, b, 1:2])
                    nc.vector.tensor_scalar_mul(tm2[:], wts[:, :, 2], att_bc[:, b, 2:3])
                    nc.vector.tensor_scalar_mul(tm3[:], wts[:, :, 3], att_bc[:, b, 3:4])
                    nc.vector.tensor_add(dst, dst, tm1[:])
                    pool_eng.tensor_add(tm2[:], tm2[:], tm3[:])
                    pool_eng.tensor_add(dst, dst, tm2[:])

        def wino_ot(b, ot):
            pb = wpool.tile([128, K_EXP], F32, tag="pb", bufs=2, name=f"pb_{b}_{ot}")
            nc.vector.tensor_mul(pb[:], bias_sb[:, ot, :], att_bc[:, b, :])
            pbr = wpool.tile([128, 1], F32, tag="pbr", bufs=2, name=f"pbr_{b}_{ot}")
            nc.vector.tensor_reduce(
                pbr[:], pb[:], axis=mybir.AxisListType.X, op=ALU.add
            )
            nc.vector.tensor_scalar(
                bias_tot[:, b, ot : ot + 1],
                pbr[:],
                inv_sb[:, ot : ot + 1],
                cnst_sb[:, ot : ot + 1],
                op0=ALU.mult,
                op1=ALU.add,
            )
            ysb = wpool.tile([128, h, w], F16, tag="ysb", bufs=1, name=f"ysb_{b}_{ot}")
            taps = list(product(range(CT), range(3)))
            wblk = WBLK
            if b == bpc - 1 and ot == 1:
                # shrink the final block so the drain->A^T->relu->DMA tail
                # after the last matmul is as short as possible
                wblk = [(0, 7), (7, 7), (14, 7), (21, 6), (27, 1)]
            coord_major = ot == 0
            msbs = {}
            if coord_major:
                # sample-0 fill: the weight bank is still streaming in
                # a-major; run every block's coord-a chain before coord a+1
                # so the PE paces the DMA arrivals instead of stalling
                # per-block on not-yet-combined coords.
                for r0, nr in wblk:
                    msbs[r0] = wpool.tile(
                        [128, 4, 7, w], F16, tag="msb", bufs=8, name=f"msb_{b}{ot}{r0}"
                    )
                for a in range(4):
                    pss = {}
                    for r0, nr in wblk:
                        pss[r0] = ppool.tile(
                            [128, 7, w], F32, tag="mps", name=f"mps_{b}{ot}{r0}_{a}"
                        )
                        for ci, (ct, kw) in enumerate(taps):
                            nc.tensor.matmul(
                                pss[r0][:, :nr, :],
                                aggs[b % 2, ct][:, a, kw, ot * 128 : (ot + 1) * 128],
                                ut[b % 2, ct][:, a, r0 : r0 + nr, 1 + kw : 1 + kw + w],
                                start=(ci == 0),
                                stop=(ci == len(taps) - 1),
                            )
                    for r0, nr in wblk:
                        nc.scalar.activation(
                            msbs[r0][:, a, :nr, :], pss[r0][:, :nr, :], AF.Copy
                        )
            for r0, nr in wblk:
                if not coord_major:
                    ps = {
                        a: ppool.tile([128, 7, w], F32, tag="mps", name=f"mps_{b}{ot}{r0}_{a}")
                        for a in range(4)
                    }
                    for a in range(4):
                        for ci, (ct, kw) in enumerate(taps):
                            nc.tensor.matmul(
                                ps[a][:, :nr, :],
                                aggs[b % 2, ct][:, a, kw, ot * 128 : (ot + 1) * 128],
                                ut[b % 2, ct][:, a, r0 : r0 + nr, 1 + kw : 1 + kw + w],
                                start=(ci == 0),
                                stop=(ci == len(taps) - 1),
                            )
                # drain M (ACT, fp32->fp16); A^T split DVE (s,y0) / Pool (d,y1);
                # fused bias+relu TSPs on DVE (4x mode)
                if coord_major:
                    msb = msbs[r0]
                else:
                    msb = wpool.tile(
                        [128, 4, 7, w], F16, tag="msb", bufs=8, name=f"msb_{b}{ot}{r0}"
                    )
                    for a in range(4):
                        nc.scalar.activation(msb[:, a, :nr, :], ps[a][:, :nr, :], AF.Copy)
                s = wpool.tile([128, 7, w], F16, tag="ys", bufs=3, name=f"ys_{b}{ot}{r0}")
                d = wpool.tile([128, 7, w], F16, tag="yd", bufs=3, name=f"yd_{b}{ot}{r0}")
                y0 = wpool.tile([128, 7, w], F16, tag="y0", bufs=3, name=f"y0_{b}{ot}{r0}")
                y1 = wpool.tile([128, 7, w], F16, tag="y1", bufs=3, name=f"y1_{b}{ot}{r0}")
                eng = nc.gpsimd if use_gpsimd_tt else nc.vector
                eng.tensor_add(s[:, :nr, :], msb[:, 1, :nr, :], msb[:, 2, :nr, :])
                eng.tensor_add(y0[:, :nr, :], msb[:, 0, :nr, :], s[:, :nr, :])
                eng.tensor_sub(d[:, :nr, :], msb[:, 1, :nr, :], msb[:, 2, :nr, :])
                eng.tensor_sub(y1[:, :nr, :], d[:, :nr, :], msb[:, 3, :nr, :])
                for r, yt in ((0, y0), (1, y1)):
                    nc.vector.tensor_scalar(
                        ysb[:, 2 * r0 + r : 2 * (r0 + nr) : 2, :],
                        yt[:, :nr, :],
                        bias_tot[:, b, ot : ot + 1],
                        0.0,
                        op0=ALU.add,
                        op1=ALU.max,
                    )
                nc.sync.dma_start(
                    out=y_d[b, ot * 128 : (ot + 1) * 128, 2 * r0 : 2 * (r0 + nr)],
                    in_=ysb[:, 2 * r0 : 2 * (r0 + nr)],
                )

        def wino_b0_fused():
            """Sample 0 with both output tiles interleaved per coord
            [ot0-a0, ot1-a0, ot0-a1, ...]: each coord's chains supply ~10us
            of PE work before the next coord's weight chunks are needed, so
            the fill runs dense against the streaming weight bank instead of
            stalling per coord."""
            b = 0
            for ot in range(OT):
                pb = wpool.tile([128, K_EXP], F32, tag="pb", bufs=2, name=f"pb_0_{ot}")
                nc.vector.tensor_mul(pb[:], bias_sb[:, ot, :], att_bc[:, b, :])
                pbr = wpool.tile([128, 1], F32, tag="pbr", bufs=2, name=f"pbr_0_{ot}")
                nc.vector.tensor_reduce(
                    pbr[:], pb[:], axis=mybir.AxisListType.X, op=ALU.add
                )
                nc.vector.tensor_scalar(
                    bias_tot[:, b, ot : ot + 1],
                    pbr[:],
                    inv_sb[:, ot : ot + 1],
                    cnst_sb[:, ot : ot + 1],
                    op0=ALU.mult,
                    op1=ALU.add,
                )
            taps = list(product(range(CT), range(3)))
            msbs = {}
            for ot, (r0, nr) in product(range(OT), WBLK):
                msbs[ot, r0] = wpool.tile(
                    [128, 4, 7, w], F16, tag="msb", bufs=8, name=f"msb_0{ot}{r0}"
                )
            for a, ot in product(range(4), range(OT)):
                pss = {}
                for r0, nr in WBLK:
                    pss[r0] = ppool.tile(
                        [128, 7, w], F32, tag="mps", name=f"mps_0{ot}{r0}_{a}"
                    )
                    if a == 0 and ot == 0 and r0 == 0:
                        # PE pstate warm-up: matmuls run 2x slower for the
                        # first 3us after idle.  A ~4us junk chain gated on
                        # u1-ct1 (~8.8us) finishes the ramp inside the fill
                        # gap so the first real chain (~12.8us) dispatches at
                        # full clock.  Each warm reads a different u slice
                        # (dedup removes identical ops) and writes the first
                        # real psum tile (unread psum chains get eliminated;
                        # the real chain's start=True resets it).
                        for i in range(12):
                            nc.tensor.matmul(
                                pss[r0][:],
                                ones_f16[:],
                                ut[0, 1][:, 1, i : i + 7, 1 : 1 + w],
                                start=True,
                                stop=True,
                            )
                    for ci, (ct, kw) in enumerate(taps):
                        nc.tensor.matmul(
                            pss[r0][:, :nr, :],
                            aggs[0, ct][:, a, kw, ot * 128 : (ot + 1) * 128],
                            ut[0, ct][:, a, r0 : r0 + nr, 1 + kw : 1 + kw + w],
                            start=(ci == 0),
                            stop=(ci == len(taps) - 1),
                        )
                for r0, nr in WBLK:
                    nc.scalar.activation(
                        msbs[ot, r0][:, a, :nr, :], pss[r0][:, :nr, :], AF.Copy
                    )
            for ot, (r0, nr) in product(range(OT), WBLK):
                msb = msbs[ot, r0]
                ysb = wpool.tile(
                    [128, h, w], F16, tag="ysb", bufs=1, name=f"ysb_0_{ot}"
                ) if r0 == 0 else ysbs[ot]
                if r0 == 0:
                    ysbs[ot] = ysb
                s = wpool.tile([128, 7, w], F16, tag="ys", bufs=3, name=f"ys_0{ot}{r0}")
                d = wpool.tile([128, 7, w], F16, tag="yd", bufs=3, name=f"yd_0{ot}{r0}")
                y0 = wpool.tile([128, 7, w], F16, tag="y0", bufs=3, name=f"y0_0{ot}{r0}")
                y1 = wpool.tile([128, 7, w], F16, tag="y1", bufs=3, name=f"y1_0{ot}{r0}")
                eng = nc.gpsimd if use_gpsimd_tt else nc.vector
                eng.tensor_add(s[:, :nr, :], msb[:, 1, :nr, :], msb[:, 2, :nr, :])
                eng.tensor_add(y0[:, :nr, :], msb[:, 0, :nr, :], s[:, :nr, :])
                eng.tensor_sub(d[:, :nr, :], msb[:, 1, :nr, :], msb[:, 2, :nr, :])
                eng.tensor_sub(y1[:, :nr, :], d[:, :nr, :], msb[:, 3, :nr, :])
                for r, yt in ((0, y0), (1, y1)):
                    nc.vector.tensor_scalar(
                        ysb[:, 2 * r0 + r : 2 * (r0 + nr) : 2, :],
                        yt[:, :nr, :],
                        bias_tot[:, b, ot : ot + 1],
                        0.0,
                        op0=ALU.add,
                        op1=ALU.max,
                    )
                nc.sync.dma_start(
                    out=y_d[b, ot * 128 : (ot + 1) * 128, 2 * r0 : 2 * (r0 + nr)],
                    in_=ysb[:, 2 * r0 : 2 * (r0 + nr)],
                )

        ysbs = {}

        # ---- schedule -------------------------------------------------------
        prep_load(0, skip_dma=True)
        prep_combine(0, fine=True)
        for b in range(bpc):
            if b + 1 < bpc:
                prep_load(b + 1)
                prep_att_head(b + 1)
                prep_combine(b + 1, fine=False)
            if b == 0:
                wino_b0_fused()
            else:
                wino_ot(b, 0)
                wino_ot(b, 1)
    nc.compile()
    return nc


G_WINO = np.array(
    [[1, 0, 0], [0.5, 0.5, 0.5], [0.5, -0.5, 0.5], [0, 0, 1]], dtype=np.float64
)


def host_inputs_wino(inputs, bpc=BPC):
    x = np.ascontiguousarray(np.asarray(inputs["x"], dtype=np.float32)).astype(
        np.float16
    )
    att_w = np.asarray(inputs["att_w"], dtype=np.float32)
    weight = np.asarray(inputs["weight"], dtype=np.float64)
    bias = np.asarray(inputs["bias"], dtype=np.float32)
    inv = np.asarray(inputs["gamma"], dtype=np.float32) / np.sqrt(
        np.asarray(inputs["run_var"], dtype=np.float32) + 1e-5
    )
    cnst = np.asarray(inputs["beta"], dtype=np.float32) - (
        np.asarray(inputs["run_mean"], dtype=np.float32) * inv
    )
    weight = weight * inv[None, :, None, None, None]
    # g[k,o,c,a,kw] = sum_kh G[a,kh] w[k,o,c,kh,kw];  wtw[i,ct,a,k,kw,o]
    g = np.einsum("ah,kochw->kocaw", G_WINO, weight)
    wtw = g.reshape(K_EXP, C_OUT, CT, 128, 4, 3).transpose(3, 2, 4, 0, 5, 1)
    wtw = np.ascontiguousarray(wtw).astype(np.float16)
    awt = np.ascontiguousarray(
        att_w.T.reshape(CT, 128, K_EXP).transpose(1, 0, 2)
    ).astype(np.float32)
    biasT = np.ascontiguousarray(
        bias.T.reshape(OT, 128, K_EXP).transpose(1, 0, 2)
    ).astype(np.float32)
    common = {
        "wtw": wtw,
        "att_wt": awt,
        "biasT": biasT,
        "bninv": inv.astype(np.float32),
        "bncnst": cnst.astype(np.float32),
    }
    return [
        {"x": x[c * bpc : (c + 1) * bpc], **common} for c in range(x.shape[0] // bpc)
    ]


WINO = True

_CACHE = {}


def _program():
    if "nc" not in _CACHE:
        import os
        kw = {}
        if os.environ.get("K_NO_GPSIMD"):
            kw["use_gpsimd_tt"] = False
        _CACHE["nc"] = build_program_wino(**kw) if WINO else build_program()
    return _CACHE["nc"]


def run(inputs, trace=False, **kw):
    nc = _program()
    in_maps = host_inputs_wino(inputs) if WINO else host_inputs(inputs)
    res = run_bass_kernel_spmd(nc, in_maps, list(range(NCORES)), trace=trace, **kw)
    y = np.concatenate(
        [res.results[c]["y"].reshape(BPC, C_OUT, H, W) for c in range(NCORES)], axis=0
    )
    return np.ascontiguousarray(y.astype(np.float32)), res


def kernel(**inputs):
    y, _ = run(inputs)
    return y

